# revision 13
# baseline (speedup 1.0000x reference)
"""Trainium2 Bass kernel for causal self-attention with RoPE.

Problem: x [1, 4096, 1024], W_qkv [3072, 1024], W_o [1024, 1024], fp32.
  qkv = x @ W_qkv.T; split Q,K,V into 16 heads of d_k=64; RoPE on Q,K;
  causal softmax(Q K^T / 8) @ V; concat heads; @ W_o.T.

Sharding: 2 heads per core across 8 cores (tensor parallel on the head dim;
W_qkv output rows and W_o input columns split per core). Each core computes a
full [4096, 1024] partial of the output projection; host sums the 8 partials.

Layout strategy (everything transposed so matmul contraction dims sit on
SBUF partitions):
  - host uploads xT [1024, 4096] and per-core W^T slices, pre-rounded to
    f32r (11-bit mantissa) so PE matmuls run at full rate with exact products
  - RoPE rotate_half is folded into extra "rotated weight" projections
    (Qr = x @ rot(Wq)^T), so Q_rot = Q*cos + Qr*sin needs no cross-partition
    shuffles
  - scores are computed transposed, St[k, q] = K Q^T, so the post-exp matrix
    E feeds the P@V matmul directly as the moving operand
  - both heads share one [128, 1024] scores psum tile so exp runs once per
    k-block over 1024 columns (amortizes the ACT per-instruction overhead)
  - V' = [V | ones]: the ones column makes row 64 of the P@V output the
    softmax denominator (no partition-axis reductions anywhere)
  - normalization (1/denom) is broadcast via a K=1 outer-product matmul and
    applied before the W_o projection
  - causal masking: fully-masked score blocks are skipped; the 4 diagonal
    staircase masks are multiplied in after exp
  - single fused loop over the 8 sequence chunks: chunk m's projections are
    immediately followed by attention for query chunk m (which only needs
    K/V chunks 0..m), so all engines stay busy end to end
"""
import numpy as np

import concourse.bacc as bacc
import concourse.mybir as mybir
import concourse.tile as tile
from concourse.bass_utils import run_bass_kernel_spmd

F32 = mybir.dt.float32
F32R = mybir.dt.float32r
AF = mybir.ActivationFunctionType

D_MODEL = 1024
N_HEADS = 16
D_K = 64
S = 4096
N_CORES = 8
HPC = N_HEADS // N_CORES      # heads per core = 2
EPC = HPC * D_K               # head dims per core = 128
CH = 512                      # s/q chunk width
NCH = S // CH                 # 8 chunks
NDT = D_MODEL // 128          # 8 contraction tiles
NSB = S // 128                # 32 s-blocks
ROPE_BASE = 10000.0


def _round_f32r(x: np.ndarray) -> np.ndarray:
    """Round fp32 to f32r (11-bit mantissa, RNE) like the PE datapath does."""
    b = np.ascontiguousarray(x, np.float32).view(np.uint32).copy()
    r = ((b >> 12) & 1).astype(np.uint32)
    b = (b + np.uint32(0x7FF) + r) & np.uint32(0xFFFFF000)
    return b.view(np.float32)


def _rope_tables():
    inv_freq = (1.0 / (ROPE_BASE ** (np.arange(0, D_K, 2, dtype=np.float32) / D_K))).astype(np.float32)
    t = np.arange(S, dtype=np.float32)
    freqs = np.outer(t, inv_freq).astype(np.float32)          # [S, 32]
    cos = np.concatenate([np.cos(freqs), np.cos(freqs)], 1)   # [S, 64]
    sin = np.concatenate([np.sin(freqs), np.sin(freqs)], 1)
    cosT = np.tile(cos.T, (HPC, 1)).astype(np.float32)        # [128, S]
    sinT = np.tile(sin.T, (HPC, 1)).astype(np.float32)
    return np.ascontiguousarray(cosT), np.ascontiguousarray(sinT)


def _masks():
    """mask[r][kk, qq] = 1 if qq >= 128*r + kk, for the 4 diagonal blocks."""
    kk = np.arange(128)[:, None]
    qq = np.arange(CH)[None, :]
    ms = [(qq >= 128 * r + kk).astype(np.float32) for r in range(4)]
    return np.ascontiguousarray(np.concatenate(ms, axis=1))   # [128, 2048]


def _rot_rows(w):
    """rotate_half applied to weight rows, per 64-row head block."""
    out = np.empty_like(w)
    for h in range(w.shape[0] // D_K):
        blk = w[h * D_K:(h + 1) * D_K]
        out[h * D_K:(h + 1) * D_K] = np.concatenate([-blk[32:], blk[:32]], axis=0)
    return out


def _build_program():
    nc = bacc.Bacc("TRN2", target_bir_lowering=False, debug=False)

    xt_d = nc.dram_tensor("xt", [D_MODEL, S], F32R, kind="ExternalInput").ap()
    wt_d = nc.dram_tensor("wt", [D_MODEL, 5 * EPC], F32R, kind="ExternalInput").ap()
    wot_d = nc.dram_tensor("wot", [EPC, D_MODEL], F32R, kind="ExternalInput").ap()
    cos_d = nc.dram_tensor("cost", [EPC, S], F32, kind="ExternalInput").ap()
    sin_d = nc.dram_tensor("sint", [EPC, S], F32, kind="ExternalInput").ap()
    msk_d = nc.dram_tensor("masks", [128, 4 * CH], F32R, kind="ExternalInput").ap()
    id_d = nc.dram_tensor("ident", [128, 128], F32, kind="ExternalInput").ap()
    y_d = nc.dram_tensor("y", [S, D_MODEL], F32, kind="ExternalOutput").ap()

    with tile.TileContext(nc) as tc:
        with tc.tile_pool(name="const", bufs=1) as cst, \
             tc.tile_pool(name="xt", bufs=12) as xtp, \
             tc.tile_pool(name="vtmp", bufs=2) as vtp, \
             tc.tile_pool(name="rope", bufs=2) as rpp, \
             tc.tile_pool(name="et", bufs=3) as etp, \
             tc.tile_pool(name="aot", bufs=2) as aop, \
             tc.tile_pool(name="ntmp", bufs=2) as ntp, \
             tc.tile_pool(name="rc", bufs=2) as rcp, \
             tc.tile_pool(name="ysb", bufs=3) as ysp, \
             tc.tile_pool(name="proj_ps", bufs=2, space="PSUM") as pjp, \
             tc.tile_pool(name="sc_ps", bufs=2, space="PSUM") as scp, \
             tc.tile_pool(name="ao_ps", bufs=1, space="PSUM") as aopp:

            # ---- persistent SBUF; DMA order matters: the first projection
            # matmuls only need wsb[d] + xt tiles, so those go out first ----
            qt_s = cst.tile([EPC, S], F32R, tag="qt")
            kt_s = cst.tile([EPC, S], F32R, tag="kt")
            vp_s = [cst.tile([128, D_K + 1], F32R, tag=f"vp{i}", name=f"vp{i}")
                    for i in range(HPC * NSB)]
            wsb = [cst.tile([128, 5 * EPC], F32R, tag=f"w{d}", name=f"w{d}")
                   for d in range(NDT)]
            cos_s = cst.tile([EPC, S], F32, tag="cos")
            sin_s = cst.tile([EPC, S], F32, tag="sin")
            xt0 = []
            for dt_i in range(NDT):
                nc.sync.dma_start(wsb[dt_i][:], wt_d[128 * dt_i:128 * (dt_i + 1), :])
                xt_t = xtp.tile([128, CH], F32R, tag="xt", name=f"xt0_{dt_i}")
                nc.sync.dma_start(xt_t[:], xt_d[128 * dt_i:128 * (dt_i + 1), 0:CH])
                xt0.append(xt_t)
            id_s = cst.tile([128, 128], F32, tag="id")
            nc.sync.dma_start(id_s[:], id_d[:])
            nc.sync.dma_start(cos_s[:, 0:CH], cos_d[:, 0:CH])
            nc.sync.dma_start(sin_s[:, 0:CH], sin_d[:, 0:CH])
            msk_s = cst.tile([128, 4 * CH], F32R, tag="msk")
            nc.sync.dma_start(msk_s[:], msk_d[:])
            wot_s = cst.tile([EPC, D_MODEL], F32R, tag="wot")
            nc.sync.dma_start(wot_s[:], wot_d[:])
            ones_f = cst.tile([128, 64], F32, tag="onesf")
            nc.vector.memset(ones_f[:], 1.0)
            ones_r = cst.tile([1, 64], F32R, tag="onesr")
            nc.vector.tensor_copy(ones_r[:], ones_f[0:1, :])

            filler = []  # fine-grained pending work items (one PE op each)

            def push_proj(m, xts_pre=None):
                """Queue projection work for s-chunk m as single-matmul items
                so the weave can fill arbitrarily small PE gaps while ACT is
                exp-bound."""
                sl = slice(CH * m, CH * (m + 1))
                xts = xts_pre if xts_pre is not None else []
                st = {}

                if xts_pre is None:
                    def dma_unit():
                        for dt_i in range(NDT):
                            xt_t = xtp.tile([128, CH], F32R, tag="xt", name=f"xt{m}_{dt_i}")
                            nc.sync.dma_start(xt_t[:], xt_d[128 * dt_i:128 * (dt_i + 1), sl])
                            xts.append(xt_t)
                        nc.sync.dma_start(cos_s[:, sl], cos_d[:, sl])
                        nc.sync.dma_start(sin_s[:, sl], sin_d[:, sl])
                    filler.append(dma_unit)

                def mm_item(b, dt_i):
                    def go():
                        if dt_i == 0:
                            st[b] = pjp.tile([128, CH], F32, tag="proj", name=f"pj{m}_{b}")
                        nc.tensor.matmul(
                            st[b][:], wsb[dt_i][:, EPC * b:EPC * (b + 1)], xts[dt_i][:],
                            start=(dt_i == 0), stop=(dt_i == NDT - 1))
                    return go

                def rope(b, dst):
                    def go():
                        ps, psr = st.pop(b), st.pop(b + 1)
                        nc.vector.tensor_mul(dst[:, sl], ps[:], cos_s[:, sl])
                        rt = rpp.tile([128, CH], F32R, tag="rt")
                        nc.vector.tensor_mul(rt[:], psr[:], sin_s[:, sl])
                        nc.vector.tensor_add(dst[:, sl], dst[:, sl], rt[:])
                    return go

                def v_evac():
                    v_ps = st.pop(4)
                    vt_t = vtp.tile([128, CH], F32, tag="vt")
                    nc.vector.tensor_copy(vt_t[:], v_ps[:])
                    st["vt"] = vt_t

                def v_tr(sb_i):
                    def go():
                        vt_t = st["vt"]
                        tr_ps = pjp.tile([128, 128], F32, tag="proj", name=f"tr{m}_{sb_i}")
                        nc.tensor.transpose(tr_ps[:], vt_t[:, 128 * sb_i:128 * (sb_i + 1)], id_s[:])
                        sb_g = (CH // 128) * m + sb_i
                        for h in range(HPC):
                            vp = vp_s[HPC * sb_g + h]
                            nc.vector.tensor_copy(vp[:, 0:D_K], tr_ps[:, D_K * h:D_K * (h + 1)])
                            nc.vector.tensor_copy(vp[:, D_K:D_K + 1], ones_f[:, 0:1])
                    return go

                for b in range(5):
                    for dt_i in range(NDT):
                        filler.append(mm_item(b, dt_i))
                    if b == 1:
                        filler.append(rope(0, qt_s))
                    elif b == 3:
                        filler.append(rope(2, kt_s))
                filler.append(v_evac)
                for sb_i in range(CH // 128):
                    filler.append(v_tr(sb_i))

            def pull(n):
                k = 0
                while filler and k < n:
                    filler.pop(0)()
                    k += 1

            # chunk 0's projections run up front (nothing to overlap with yet)
            push_proj(0, xts_pre=xt0)
            pull(len(filler))

            for m in range(NCH):
                j = m
                qsl = slice(CH * j, CH * (j + 1))
                nkb = (CH // 128) * (j + 1)
                if m + 1 < NCH:
                    push_proj(m + 1)
                total = len(filler)
                done = 0

                ao = [aopp.tile([D_K + 1, CH], F32, tag=f"ao{h}", name=f"ao{j}_{h}")
                      for h in range(HPC)]
                for kb in range(nkb):
                    sc_t = scp.tile([128, 2 * CH], F32, tag="sc", name=f"sc{j}_{kb}")
                    for h in range(HPC):
                        nc.tensor.matmul(
                            sc_t[:, CH * h:CH * (h + 1)],
                            kt_s[D_K * h:D_K * (h + 1), 128 * kb:128 * (kb + 1)],
                            qt_s[D_K * h:D_K * (h + 1), qsl],
                            start=True, stop=True, tile_position=(D_K * h, 0))
                    et_t = etp.tile([128, 2 * CH], F32R, tag="et", name=f"et{j}_{kb}")
                    nc.scalar.activation(et_t[:], sc_t[:], AF.Exp, scale=0.125)
                    r = kb - (CH // 128) * j
                    if r >= 0:
                        for h in range(HPC):
                            nc.vector.tensor_mul(
                                et_t[:, CH * h:CH * (h + 1)],
                                et_t[:, CH * h:CH * (h + 1)],
                                msk_s[:, CH * r:CH * (r + 1)])
                    for h in range(HPC):
                        nc.tensor.matmul(
                            ao[h][:], vp_s[HPC * kb + h][:], et_t[:, CH * h:CH * (h + 1)],
                            start=(kb == 0), stop=(kb == nkb - 1))
                    # weave next chunk's projection work into the exp-bound
                    # loop, holding a few items back for the normalize stretch
                    want = min(max(total - 6, 0), total * (kb + 1) // nkb)
                    pull(want - done)
                    done = want

                # ---- normalize and assemble AO^T [128, CH] ----
                aot_t = aop.tile([EPC, CH], F32R, tag="aot")
                for h in range(HPC):
                    rc_t = rcp.tile([1, CH], F32R, tag="rc")
                    with nc.allow_low_precision("softmax denom reciprocal"):
                        nc.vector.reciprocal(rc_t[:], ao[h][D_K:D_K + 1, :])
                    bc_t = aopp.tile([D_K, CH], F32, tag=f"ao{h}", name=f"bc{j}_{h}")
                    nc.tensor.matmul(bc_t[:], ones_r[:], rc_t[:], start=True, stop=True)
                    nt_t = ntp.tile([D_K, CH], F32, tag="nt")
                    nc.scalar.copy(nt_t[:], ao[h][0:D_K, :])
                    nc.vector.tensor_mul(aot_t[D_K * h:D_K * (h + 1), :], nt_t[:], bc_t[:])
                    pull(3)

                pull(len(filler))

                # ---- W_o partial: y[q, :] = AO^T.T @ WoT ----
                for sb_i in range(CH // 128):
                    for half in range(2):
                        y_ps = scp.tile([128, 512], F32, tag="sc",
                                        name=f"y{j}_{sb_i}_{half}")
                        nc.tensor.matmul(
                            y_ps[:], aot_t[:, 128 * sb_i:128 * (sb_i + 1)],
                            wot_s[:, 512 * half:512 * (half + 1)],
                            start=True, stop=True)
                        y_sb = ysp.tile([128, 512], F32, tag="y")
                        nc.scalar.copy(y_sb[:], y_ps[:])
                        nc.sync.dma_start(
                            y_d[CH * j + 128 * sb_i:CH * j + 128 * (sb_i + 1),
                                512 * half:512 * (half + 1)],
                            y_sb[:])
    nc.compile()
    return nc


_PROGRAM = None


def _prep_inputs(x, W_qkv, W_o):
    x2 = np.ascontiguousarray(x.reshape(S, D_MODEL), np.float32)
    xt = _round_f32r(x2.T)
    cosT, sinT = _rope_tables()
    masks = _masks()
    ident = np.eye(128, dtype=np.float32)
    in_maps = []
    for c in range(N_CORES):
        rows = slice(EPC * c, EPC * (c + 1))
        wq = W_qkv[0 * D_MODEL:1 * D_MODEL][rows]
        wk = W_qkv[1 * D_MODEL:2 * D_MODEL][rows]
        wv = W_qkv[2 * D_MODEL:3 * D_MODEL][rows]
        wt = np.concatenate([wq, _rot_rows(wq), wk, _rot_rows(wk), wv], axis=0).T
        wot = W_o[:, rows].T
        in_maps.append({
            "xt": xt,
            "wt": _round_f32r(np.ascontiguousarray(wt)),
            "wot": _round_f32r(np.ascontiguousarray(wot)),
            "cost": cosT,
            "sint": sinT,
            "masks": masks,
            "ident": ident,
        })
    return in_maps


def kernel(x, W_qkv, W_o):
    global _PROGRAM
    x = np.asarray(x, np.float32)
    W_qkv = np.asarray(W_qkv, np.float32)
    W_o = np.asarray(W_o, np.float32)
    if _PROGRAM is None:
        _PROGRAM = _build_program()
    in_maps = _prep_inputs(x, W_qkv, W_o)
    res = run_bass_kernel_spmd(_PROGRAM, in_maps, core_ids=list(range(N_CORES)))
    acc = np.zeros((S, D_MODEL), np.float64)
    for r in res.results:
        acc += r["y"].astype(np.float64)
    return acc.astype(np.float32).reshape(1, S, D_MODEL)
